# revision 29
# baseline (speedup 1.0000x reference)
"""Trainium2 Bass kernel for one Mixtral-style layer (nn_MixtralModel).

Self-contained: hardcodes shapes from the problem spec.
  T=2048 tokens, H=1024 hidden, 16 Q heads / 4 KV heads, D=64, RoPE neox,
  causal GQA attention, MoE E=8 experts top-2, I=2048 intermediate.

Sharding across 8 NeuronCores (tensor-parallel):
  - attention: 2 Q heads + their shared KV head per core; chunked AllGather
    of head outputs; o_proj column-parallel (each core computes a 128-row
    slice).
  - norms: per-feature norm weights folded into downstream matmul weights on
    the host; per-token 1/rms applied on-device via PE outer-product
    broadcasts; cross-core sum-of-squares via tiny AllReduce.
  - gate: partial logits from each core's 128-feature slice of the residual,
    summed in the same AllReduce as the norm sum-of-squares, scaled by the
    1/rms broadcast afterwards (scale commutes with the cross-core sum).
  - MoE: intermediate dim sharded (256 per expert per core), dense compute of
    all experts, chunked ReduceScatter of the partial outputs.

All collectives are split into 4 token-chunks so they overlap compute.
Everything on device is feature-major: activations [features, T] with
features on SBUF partitions.
"""
import os
import numpy as np
import ml_dtypes

import concourse.bass as bass
import concourse.bacc as bacc
import concourse.mybir as mybir
import concourse.tile as tile
from concourse.bass_utils import run_bass_kernel_spmd

F32 = mybir.dt.float32
BF16 = mybir.dt.bfloat16
NC_N = 8
T = 2048
H = 1024
HQ, HK, D = 16, 4, 64
E = 8
I = 2048
IS = I // NC_N          # 256
EPS = 1e-5
THETA = 10000.0
P = 128
TCH = 512               # free-dim chunk (one fp32 PSUM bank)
NCH = T // TCH          # 4
NKH = H // P            # 8 k-tiles over hidden
NEG = -1.0e9
AF = mybir.ActivationFunctionType
ALU = mybir.AluOpType

MMDT_NAME = os.environ.get("KB_MM_DT", "f32r")   # f32 | f32r
ADT_NAME = os.environ.get("KB_A_DT", MMDT_NAME)
MDT_NAME = os.environ.get("KB_M_DT", MMDT_NAME)
SDT_NAME = os.environ.get("KB_S_DT", MMDT_NAME)
RDT_NAME = os.environ.get("KB_R_DT", MDT_NAME)   # routing-weight plumbing
W13DT_NAME = os.environ.get("KB_W13_DT", MDT_NAME)
W2DT_NAME = os.environ.get("KB_W2_DT", MDT_NAME)


_DTM = {"f32": mybir.dt.float32, "f32r": mybir.dt.float32r,
        "bf16": mybir.dt.bfloat16}
MMDT = _DTM[MMDT_NAME]
ADT = _DTM[ADT_NAME]
MDT = _DTM[MDT_NAME]
SDT = _DTM[SDT_NAME]
RDT = _DTM[RDT_NAME]
W13DT = _DTM[W13DT_NAME]
W2DT = _DTM[W2DT_NAME]


def build_program():
    nc = bacc.Bacc("TRN2", target_bir_lowering=False, debug=False,
                   num_devices=NC_N)

    def inp(name, shape):
        return nc.dram_tensor(name, shape, F32, kind="ExternalInput")

    def inp_a(name, shape):
        return nc.dram_tensor(name, shape, ADT, kind="ExternalInput")

    def inp_s(name, shape):
        return nc.dram_tensor(name, shape, SDT, kind="ExternalInput")

    x_fm = inp_a("x_fm", [H, T])
    x_sl = inp("x_sl", [P, T])
    wqkv = inp_a("wqkv", [H, 256])       # q rows pre-scaled by 1/sqrt(D)
    wo = inp_a("wo", [H, P])
    cos_q = inp("cos_q", [P, T])
    sin_q = inp("sin_q", [P, T])
    qswap = inp_a("qswap", [P, P])
    kswap = inp_a("kswap", [64, 64])
    kdup = inp_a("kdup", [64, P])
    ident = inp("ident", [P, P])
    ones_c = inp_s("ones_c", [P, 1])
    ones_r = inp_s("ones_r", [1, P])
    dmask = inp("dmask", [4, P, TCH])
    gwl = inp("gwl", [P, E])             # this core's gate_w.T slice
    esel = nc.dram_tensor("esel", [E, E * P], RDT, kind="ExternalInput")
    w13 = nc.dram_tensor("w13", [E, H, 2 * IS], W13DT, kind="ExternalInput")
    w2T = nc.dram_tensor("w2T", [NKH, P, 2 * E * P], W2DT,
                         kind="ExternalInput")
    wnext = inp("wnext", [P, 1])
    out_sl = nc.dram_tensor("out_sl", [P, T], F32, kind="ExternalOutput")

    RG = [list(range(NC_N))]

    with tile.TileContext(nc) as tc:
        with (
            tc.tile_pool(name="dram", bufs=1, space="DRAM") as dram,
            tc.tile_pool(name="persist", bufs=1) as pp,
            tc.tile_pool(name="smalls", bufs=1) as sp,
            tc.tile_pool(name="vecs", bufs=2) as vp,
        ):
            ag_att_in = [dram.tile([P, TCH], ADT, name=f"agAi{n}",
                                   tag=f"b0_{n}") for n in range(NCH)]
            ag_att_out = [dram.tile([H, TCH], ADT, addr_space="Shared",
                                    name=f"agAo{n}", tag=f"b1_{n}")
                          for n in range(NCH)]
            ar2_in = [dram.tile([1 + E, TCH], F32, name=f"ar2i{n}",
                                tag=f"b2_{n}") for n in range(NCH)]
            ar2_out = [dram.tile([1 + E, TCH], F32, addr_space="Shared",
                                 name=f"ar2o{n}", tag=f"b3_{n}")
                       for n in range(NCH)]
            ag_h_in = [dram.tile([P, TCH], F32, name=f"agHi{n}",
                                 tag=f"b4_{n}") for n in range(NCH)]
            ag_h_out = [dram.tile([H, TCH], F32, addr_space="Shared",
                                  name=f"agHo{n}", tag=f"b5_{n}")
                        for n in range(NCH)]
            rs_in = [dram.tile([H, TCH], F32, name=f"rsi{n}",
                               tag=f"b6_{n}") for n in range(NCH)]
            rs_out = [dram.tile([P, TCH], F32, name=f"rso{n}",
                                tag=f"b7_{n}") for n in range(NCH)]
            ar3_in = dram.tile([1, T], F32, tag="b8")
            ar3_out = dram.tile([1, T], F32, addr_space="Shared", tag="b9")

            xsl_t = pp.tile([P, T], F32, tag="xsl")
            resid = pp.tile([P, T], F32, tag="resid")

            onec_t = sp.tile([P, 1], SDT, tag="onec")
            oner_t = sp.tile([1, P], SDT, tag="oner")
            ident_ta = sp.tile([P, P], ADT, tag="identa")
            ident_tm = sp.tile([P, P], RDT, tag="identm")
            ident_tf = sp.tile([P, P], F32, tag="identf")
            wnext_t = sp.tile([P, 1], F32, tag="wnext")
            gwl_t = sp.tile([P, E], F32, tag="gwl")
            l0s = sp.tile([E, T], F32, tag="l0s")
            nc.sync.dma_start(onec_t[:], ones_c[:])
            nc.sync.dma_start(oner_t[:], ones_r[:])
            nc.sync.dma_start(ident_ta[:], ident[:].bitcast(ADT))
            nc.sync.dma_start(ident_tm[:], ident[:].bitcast(RDT))
            nc.sync.dma_start(ident_tf[:], ident[:])
            nc.sync.dma_start(wnext_t[:], wnext[:])
            nc.sync.dma_start(gwl_t[:], gwl[:])

            def rms_inv_chunk(ss_ap, tagn):
                """[1,TCH] chunk of sumsq -> [1,TCH] 1/rms tile."""
                rmst = vp.tile([1, TCH], F32, name=f"rms{tagn}", tag="rmsv")
                nc.vector.tensor_scalar(rmst[:], ss_ap, 1.0 / H, EPS,
                                        op0=ALU.mult, op1=ALU.add)
                nc.scalar.activation(rmst[:], rmst[:], AF.Sqrt)
                invt = vp.tile([1, TCH], SDT, name=f"inv{tagn}", tag="invv")
                with nc.allow_low_precision(reason="f32r rounding of 1/rms"):
                    nc.vector.reciprocal(invt[:], rmst[:])
                return invt

            # ======== attention region ========
            with tc.tile_pool(name="apool", bufs=1) as apool:
                q_rope = apool.tile([P, T], ADT, tag="qrope")
                k_dup = apool.tile([P, T], ADT, tag="kdup")
                v_tm = apool.tile([P, (T // P) * 65], ADT, tag="vtm")

                with (
                    tc.tile_pool(name="xchunk", bufs=1) as xcp,
                    tc.tile_pool(name="wqkvp", bufs=1) as wqp,
                    tc.tile_pool(name="sqp", bufs=2) as sqp,
                    tc.tile_pool(name="qkvsb", bufs=1) as qkvp,
                    tc.tile_pool(name="cossin", bufs=1) as csp,
                    tc.tile_pool(name="ropetmp", bufs=2) as rtp,
                    tc.tile_pool(name="psA", bufs=1, space="PSUM") as psA,
                ):
                    wqt = [wqp.tile([P, 256], ADT, name=f"wqt{k}",
                                    tag=f"wqt{k}") for k in range(NKH)]
                    for k in range(NKH):
                        nc.sync.dma_start(wqt[k][:], wqkv[P * k:P * (k + 1), :])
                    cq = csp.tile([P, T], F32, tag="cq")
                    sq_ = csp.tile([P, T], F32, tag="sq_")
                    qsw = csp.tile([P, P], ADT, tag="qsw")
                    ksw = csp.tile([64, 64], ADT, tag="ksw")
                    kdp = csp.tile([64, P], ADT, tag="kdp")


                    qkv_sb = [qkvp.tile([P, T], ADT, name=f"qkv{m}",
                                        tag=f"qkv{m}") for m in range(2)]
                    first_dma_done = False

                    # per-chunk: x load -> sumsq -> invrms -> qkv -> rope
                    for n in range(NCH):
                        c0, c1 = TCH * n, TCH * (n + 1)
                        xc = [xcp.tile([P, TCH], ADT, name=f"xc{n}_{k}",
                                       tag=f"xc{k}", bufs=2)
                              for k in range(NKH)]
                        for k in range(NKH):
                            nc.sync.dma_start(xc[k][:],
                                              x_fm[P * k:P * (k + 1), c0:c1])
                        if not first_dma_done:
                            # bulky loads not needed until RoPE / o_proj go
                            # behind the first compute chunk's inputs
                            first_dma_done = True
                            nc.sync.dma_start(cq[:], cos_q[:])
                            nc.sync.dma_start(sq_[:], sin_q[:])
                            nc.sync.dma_start(qsw[:], qswap[:])
                            nc.sync.dma_start(ksw[:], kswap[:])
                            nc.sync.dma_start(kdp[:], kdup[:])
                            nc.sync.dma_start(xsl_t[:], x_sl[:])
                        ssp = psA.tile([1, TCH], F32, name=f"ssp{n}",
                                       tag="pA", bufs=6)
                        for k in range(NKH):
                            sq = sqp.tile([P, TCH], SDT, name=f"sqx{n}_{k}",
                                          tag="sqx")
                            nc.scalar.activation(sq[:], xc[k][:], AF.Square)
                            nc.tensor.matmul(ssp[:], onec_t[:], sq[:],
                                             start=(k == 0),
                                             stop=(k == NKH - 1))
                        ssv = vp.tile([1, TCH], F32, name=f"ssv{n}", tag="ssv")
                        nc.scalar.copy(ssv[:], ssp[:])
                        invt = rms_inv_chunk(ssv[:], f"1_{n}")
                        bc = psA.tile([P, TCH], F32, name=f"bc{n}", tag="bc",
                                      bufs=2)
                        nc.tensor.matmul(bc[:], oner_t[:], invt[:],
                                         start=True, stop=True)
                        bcs = sqp.tile([P, TCH], F32, name=f"bcs{n}",
                                       tag="bcs")
                        nc.scalar.copy(bcs[:], bc[:])
                        for m in range(2):
                            qp = psA.tile([P, TCH], F32, name=f"qp{m}_{n}",
                                          tag="pA", bufs=6)
                            for k in range(NKH):
                                nc.tensor.matmul(
                                    qp[:], wqt[k][:, P * m:P * (m + 1)],
                                    xc[k][:], start=(k == 0),
                                    stop=(k == NKH - 1))
                            nc.vector.tensor_mul(qkv_sb[m][:, c0:c1], qp[:],
                                                 bcs[:])
                        # RoPE on q (2 heads) and k; k duplicated to both
                        # partition halves; v transposed below.
                        qs = psA.tile([P, TCH], F32, name=f"qs{n}",
                                      tag="pA", bufs=6)
                        nc.tensor.matmul(qs[:], qsw[:], qkv_sb[0][:, c0:c1],
                                         start=True, stop=True)
                        t1 = rtp.tile([P, TCH], F32, name=f"rt{n}", tag="rt")
                        nc.vector.tensor_mul(t1[:], qs[:], sq_[:, c0:c1])
                        nc.vector.tensor_mul(q_rope[:, c0:c1],
                                             qkv_sb[0][:, c0:c1],
                                             cq[:, c0:c1])
                        nc.vector.tensor_add(q_rope[:, c0:c1],
                                             q_rope[:, c0:c1], t1[:])
                        ks_ = psA.tile([64, TCH], F32, name=f"ks{n}",
                                       tag="pA", bufs=6)
                        nc.tensor.matmul(ks_[:], ksw[:],
                                         qkv_sb[1][0:64, c0:c1],
                                         start=True, stop=True)
                        t2 = rtp.tile([64, TCH], F32, name=f"rt2_{n}",
                                      tag="rt2")
                        nc.vector.tensor_mul(t2[:], ks_[:],
                                             sq_[0:64, c0:c1])
                        k_tmp = rtp.tile([64, TCH], ADT, name=f"kt{n}",
                                         tag="kt")
                        nc.vector.tensor_mul(k_tmp[:],
                                             qkv_sb[1][0:64, c0:c1],
                                             cq[0:64, c0:c1])
                        nc.vector.tensor_add(k_tmp[:], k_tmp[:], t2[:])
                        kd_ps = psA.tile([P, TCH], F32, name=f"kd{n}",
                                         tag="pA", bufs=6)
                        nc.tensor.matmul(kd_ps[:], kdp[:], k_tmp[:],
                                         start=True, stop=True)
                        nc.scalar.copy(k_dup[:, c0:c1], kd_ps[:])
                        for ii in range(4):
                            i = 4 * n + ii
                            vt = psA.tile([P, 64], ADT, name=f"vtp{i}",
                                          tag="pA", bufs=6)
                            nc.tensor.transpose(
                                vt[:], qkv_sb[1][64:128, P * i:P * (i + 1)],
                                ident_ta[64:128, 64:128])
                            nc.scalar.copy(v_tm[:, 65 * i:65 * i + 64], vt[:])
                            nc.gpsimd.memset(
                                v_tm[:, 65 * i + 64:65 * i + 65].bitcast(F32),
                                1.0)

                # ---- attention scores/softmax/PV, chunked AllGather ----
                with (
                    tc.tile_pool(name="pt", bufs=4) as ptp,
                    tc.tile_pool(name="dmaskp", bufs=1) as dmp,
                    tc.tile_pool(name="dsb", bufs=2) as dsb,
                    tc.tile_pool(name="attc", bufs=2) as atc,
                    tc.tile_pool(name="psD", bufs=1, space="PSUM") as psD,
                ):
                    dm = [dmp.tile([P, TCH], F32, name=f"dm{m}", tag=f"dm{m}")
                          for m in range(4)]
                    for m in range(4):
                        nc.sync.dma_start(dm[m][:], dmask[m])
                    for n in range(NCH):
                        c0, c1 = TCH * n, TCH * (n + 1)
                        attn_c = atc.tile([P, TCH], ADT, name=f"attc{n}",
                                          tag="attc")
                        for h in range(2):
                            qh = q_rope[64 * h:64 * h + 64, :]
                            kh = k_dup[64 * h:64 * h + 64, :]
                            ap_ = psD.tile([65, TCH], F32, name=f"ap{h}_{n}",
                                           tag="ap", bufs=2)
                            jmax = 4 * n + 3
                            p_ts = {}

                            def emit_score(j):
                                s_ps = psD.tile([P, TCH], F32,
                                                name=f"s{h}_{n}_{j}",
                                                tag="s", bufs=3)
                                nc.tensor.matmul(
                                    s_ps[:],
                                    kh[:, P * j:P * (j + 1)],
                                    qh[:, c0:c1],
                                    start=True, stop=True)
                                if j >= 4 * n:
                                    nc.vector.tensor_add(s_ps[:], s_ps[:],
                                                         dm[j - 4 * n][:])
                                p_t = ptp.tile([P, TCH], ADT,
                                               name=f"p{h}_{n}_{j}", tag="p")
                                nc.scalar.activation(p_t[:], s_ps[:], AF.Exp)
                                p_ts[j] = p_t

                            # scores run 2 blocks ahead of PV so the PE never
                            # waits on the vector-add + exp dependency chain
                            emit_score(0)
                            if jmax >= 1:
                                emit_score(1)
                            for j in range(jmax + 1):
                                if j + 2 <= jmax:
                                    emit_score(j + 2)
                                nc.tensor.matmul(
                                    ap_[:], v_tm[:, 65 * j:65 * (j + 1)],
                                    p_ts.pop(j)[:],
                                    start=(j == 0), stop=(j == jmax))
                            isum = dsb.tile([1, TCH], SDT, name=f"is{h}{n}",
                                            tag="is")
                            with nc.allow_low_precision(
                                    reason="f32r rounding of 1/rowsum"):
                                nc.vector.reciprocal(isum[:], ap_[64:65, :])
                            bc = psD.tile([64, TCH], F32, name=f"abc{h}{n}",
                                          tag="abc", bufs=1)
                            nc.tensor.matmul(bc[:], oner_t[0:1, 0:64],
                                             isum[:], start=True, stop=True)
                            bcs = dsb.tile([64, TCH], F32, name=f"abcs{h}{n}",
                                           tag="abcs")
                            nc.scalar.copy(bcs[:], bc[:])
                            nc.vector.tensor_mul(
                                attn_c[64 * h:64 * h + 64, :],
                                ap_[0:64, :], bcs[:])
                        nc.sync.dma_start(ag_att_in[n][:], attn_c[:])
                        nc.gpsimd.collective_compute(
                            "AllGather", ALU.bypass, replica_groups=RG,
                            ins=[ag_att_in[n].opt()],
                            outs=[ag_att_out[n].opt()])

            # ======== o_proj + residual + sumsq&gate partials (chunked) ====
            with (
                tc.tile_pool(name="attc2", bufs=1) as acp,
                tc.tile_pool(name="wop", bufs=1) as wop,
                tc.tile_pool(name="sqf", bufs=2) as sqf,
                tc.tile_pool(name="hslp", bufs=1) as hslp,
                tc.tile_pool(name="arsb", bufs=2) as arp,
                tc.tile_pool(name="psF", bufs=1, space="PSUM") as psF,
            ):
                wot = [wop.tile([P, P], ADT, name=f"wot{k}", tag=f"wot{k}")
                       for k in range(NKH)]
                for k in range(NKH):
                    nc.sync.dma_start(wot[k][:], wo[P * k:P * (k + 1), :])
                for n in range(NCH):
                    c0, c1 = TCH * n, TCH * (n + 1)
                    ac = [acp.tile([P, TCH], ADT, name=f"ac{n}_{k}",
                                   tag=f"ac{k}", bufs=2)
                          for k in range(NKH)]
                    for k in range(NKH):
                        nc.sync.dma_start(
                            ac[k][:], ag_att_out[n][P * k:P * (k + 1), :])
                    op_ = psF.tile([P, TCH], F32, name=f"op{n}", tag="op",
                                   bufs=2)
                    for k in range(NKH):
                        nc.tensor.matmul(op_[:], wot[k][:], ac[k][:],
                                         start=(k == 0), stop=(k == NKH - 1))
                    nc.vector.tensor_add(resid[:, c0:c1], op_[:],
                                         xsl_t[:, c0:c1])
                    sq2 = sqf.tile([P, TCH], SDT, name=f"sq2_{n}", tag="sq2")
                    nc.scalar.activation(sq2[:], resid[:, c0:c1], AF.Square)
                    ssp2 = psF.tile([1, TCH], F32, name=f"ss2p{n}",
                                    tag="ss2p", bufs=2)
                    nc.tensor.matmul(ssp2[:], onec_t[:], sq2[:],
                                     start=True, stop=True)
                    ss2sb = arp.tile([1, TCH], F32, name=f"s2s{n}",
                                     tag="s2s")
                    nc.scalar.copy(ss2sb[:], ssp2[:])
                    # partial gate logits from this core's feature slice
                    # (full f32: routing top-2 flips under f32r rounding)
                    l0p = psF.tile([E, TCH], F32, name=f"l0p{n}", tag="l0",
                                   bufs=2)
                    nc.tensor.matmul(l0p[:], gwl_t[:], resid[:, c0:c1],
                                     start=True, stop=True)
                    l0sb = arp.tile([E, TCH], F32, name=f"l0c{n}", tag="l0c")
                    nc.scalar.copy(l0sb[:], l0p[:])
                    nc.sync.dma_start(ar2_in[n][0:1, :], ss2sb[:])
                    nc.sync.dma_start(ar2_in[n][1:1 + E, :], l0sb[:])
                    nc.gpsimd.collective_compute(
                        "AllReduce", ALU.add, replica_groups=RG,
                        ins=[ar2_in[n].opt()], outs=[ar2_out[n].opt()])

                # hsl + scaled gate logits per chunk; chunked h AllGather

                for n in range(NCH):
                    c0, c1 = TCH * n, TCH * (n + 1)
                    ss2g = arp.tile([1, TCH], F32, name=f"s2g{n}",
                                    tag="s2g")
                    l0g = arp.tile([E, TCH], F32, name=f"l0g{n}", tag="l0g")
                    nc.sync.dma_start(ss2g[:], ar2_out[n][0:1, :])
                    nc.sync.dma_start(l0g[:], ar2_out[n][1:1 + E, :])
                    invt = rms_inv_chunk(ss2g[:], f"2_{n}")
                    bc = psF.tile([P, TCH], F32, name=f"n2bc{n}", tag="n2bc",
                                  bufs=2)
                    nc.tensor.matmul(bc[:], oner_t[:], invt[:],
                                     start=True, stop=True)
                    hslc = hslp.tile([P, TCH], F32, name=f"hsl{n}",
                                     tag="hslc", bufs=2)
                    nc.vector.tensor_mul(hslc[:], bc[:], resid[:, c0:c1])
                    nc.vector.tensor_mul(l0s[:, c0:c1], bc[0:E, :],
                                         l0g[:])
                    nc.sync.dma_start(ag_h_in[n][:], hslc[:])
                    nc.gpsimd.collective_compute(
                        "AllGather", ALU.bypass, replica_groups=RG,
                        ins=[ag_h_in[n].opt()], outs=[ag_h_out[n].opt()])

            # ======== routing + MoE, per T-quarter; chunked RS ========
            with (
                tc.tile_pool(name="hq", bufs=1) as hqp,
                tc.tile_pool(name="gwp", bufs=1) as gwp,
                tc.tile_pool(name="routp", bufs=2) as rp,
                tc.tile_pool(name="wpool", bufs=1) as wp,
                tc.tile_pool(name="gatedp", bufs=1) as gp,
                tc.tile_pool(name="wbcp", bufs=2) as wbp,
                tc.tile_pool(name="silup", bufs=2) as slp,
                tc.tile_pool(name="moesb", bufs=3) as msb,
                tc.tile_pool(name="psM", bufs=1, space="PSUM") as psM,
            ):
                esel_t = gwp.tile([E, E * P], RDT, tag="esel")
                nc.sync.dma_start(esel_t[:], esel[:])
                NTQ = TCH // P  # 4 token-tiles per quarter


                for tq in range(NCH):
                    t0 = TCH * tq
                    hh = [hqp.tile([P, TCH], F32, name=f"hh{tq}_{k}",
                                   tag=f"hh{k}", bufs=2) for k in range(NKH)]
                    for k in range(NKH):
                        nc.sync.dma_start(
                            hh[k][:], ag_h_out[tq][P * k:P * (k + 1), :])


                    # token-major gate logits via transposes of l0s
                    LT = rp.tile([P, NTQ * E], F32, name=f"LT{tq}", tag="LT")
                    for i in range(NTQ):
                        lg = psM.tile([P, E], F32, name=f"lg{tq}_{i}",
                                      tag="pM1", bufs=2)
                        nc.tensor.transpose(
                            lg[:], l0s[:, t0 + P * i:t0 + P * (i + 1)],
                            ident_tf[0:E, 0:E])
                        nc.scalar.copy(LT[:, E * i:E * (i + 1)], lg[:])
                    LT3 = LT[:].rearrange("p (i e) -> p i e", e=E)
                    m1 = rp.tile([P, NTQ], F32, name=f"m1{tq}", tag="m1")
                    nc.vector.reduce_max(m1[:], LT3, axis=mybir.AxisListType.X)
                    eq1 = rp.tile([P, NTQ * E], F32, name=f"eq1{tq}",
                                  tag="eq1")
                    eq13 = eq1[:].rearrange("p (i e) -> p i e", e=E)
                    nc.vector.tensor_tensor(
                        eq13, LT3, m1[:, :, None].to_broadcast((P, NTQ, E)),
                        op=ALU.is_equal)
                    tmp = rp.tile([P, NTQ * E], F32, name=f"tmp{tq}",
                                  tag="tmpr")
                    tmp3 = tmp[:].rearrange("p (i e) -> p i e", e=E)
                    nc.vector.tensor_single_scalar(tmp3, eq13, 1.0e30,
                                                   op=ALU.mult)
                    lm = rp.tile([P, NTQ * E], F32, name=f"lm{tq}", tag="lm")
                    lm3 = lm[:].rearrange("p (i e) -> p i e", e=E)
                    nc.vector.tensor_sub(lm3, LT3, tmp3)
                    m2 = rp.tile([P, NTQ], F32, name=f"m2{tq}", tag="m2")
                    nc.vector.reduce_max(m2[:], lm3, axis=mybir.AxisListType.X)
                    eq2 = rp.tile([P, NTQ * E], F32, name=f"eq2{tq}",
                                  tag="eq2")
                    eq23 = eq2[:].rearrange("p (i e) -> p i e", e=E)
                    nc.vector.tensor_tensor(
                        eq23, lm3, m2[:, :, None].to_broadcast((P, NTQ, E)),
                        op=ALU.is_equal)
                    dmx = rp.tile([P, NTQ], F32, name=f"dmx{tq}", tag="dmx")
                    nc.vector.tensor_sub(dmx[:], m2[:], m1[:])
                    qe = rp.tile([P, NTQ], F32, name=f"qe{tq}", tag="qe")
                    nc.scalar.activation(qe[:], dmx[:], AF.Exp)
                    den = rp.tile([P, NTQ], F32, name=f"den{tq}", tag="den")
                    nc.vector.tensor_single_scalar(den[:], qe[:], 1.0,
                                                   op=ALU.add)
                    inv = rp.tile([P, NTQ], F32, name=f"invr{tq}", tag="invr")
                    nc.vector.reciprocal(inv[:], den[:])
                    qinv = rp.tile([P, NTQ], F32, name=f"qinv{tq}", tag="qinv")
                    nc.vector.tensor_mul(qinv[:], qe[:], inv[:])
                    wd = rp.tile([P, NTQ * E], RDT, name=f"wd{tq}", tag="wd")
                    wd3 = wd[:].rearrange("p (i e) -> p i e", e=E)
                    nc.vector.tensor_mul(
                        wd3, eq13, inv[:, :, None].to_broadcast((P, NTQ, E)))
                    nc.vector.tensor_mul(
                        eq23, eq23,
                        qinv[:, :, None].to_broadcast((P, NTQ, E)))
                    nc.vector.tensor_add(wd3, wd3, eq23)
                    wd_fm = rp.tile([E, TCH], RDT, name=f"wdfm{tq}",
                                    tag="wdfm")
                    for i in range(NTQ):
                        tp = psM.tile([E, P], RDT, name=f"wdt{tq}_{i}",
                                      tag="pM1", bufs=2)
                        nc.tensor.transpose(tp[:], wd[:, E * i:E * (i + 1)],
                                            ident_tm[:])
                        nc.scalar.copy(wd_fm[:, P * i:P * (i + 1)], tp[:])

                    # w1/w3 phase
                    gt = {}
                    for e in range(E):
                        for m in range(2):
                            gt[(e, m)] = gp.tile(
                                [P, TCH], W2DT, name=f"g{tq}_{e}_{m}",
                                tag=f"g{e}_{m}")
                    for e in range(E):
                        wt13 = [wp.tile([P, 2 * TCH], W13DT,
                                        name=f"w13_{tq}_{e}_{kk}",
                                        tag=f"w13_{kk}", bufs=2)
                                for kk in range(4)]
                        src_ap = w13[e].rearrange(
                            "(kk two p) i -> kk p two i", p=P, two=2)
                        for kk in range(4):
                            nc.sync.dma_start(
                                wt13[kk][:].rearrange(
                                    "p (two i) -> p two i", two=2),
                                src_ap[kk])
                        wb = psM.tile([P, TCH], F32, name=f"wb{tq}{e}",
                                      tag="pM1", bufs=2)
                        nc.tensor.matmul(wb[:], esel_t[:, P * e:P * (e + 1)],
                                         wd_fm[:], start=True, stop=True)
                        wbs = wbp.tile([P, TCH], F32, name=f"wbs{tq}{e}",
                                       tag="wbs")
                        nc.scalar.copy(wbs[:], wb[:])
                        for m in range(2):
                            h1p = psM.tile([P, TCH], F32,
                                           name=f"h1_{tq}{e}{m}", tag="h1",
                                           bufs=2)
                            h3p = psM.tile([P, TCH], F32,
                                           name=f"h3_{tq}{e}{m}", tag="h3",
                                           bufs=2)
                            for k in range(NKH):
                                kk, tt = k // 2, k % 2
                                w1ap = wt13[kk][:, TCH * tt + P * m:
                                                TCH * tt + P * m + P]
                                nc.tensor.matmul(
                                    h1p[:], w1ap,
                                    hh[k][:].bitcast(W13DT),
                                    start=(k == 0), stop=(k == NKH - 1))
                            for k in range(NKH):
                                kk, tt = k // 2, k % 2
                                w3ap = wt13[kk][:, TCH * tt + 2 * P + P * m:
                                                TCH * tt + 2 * P + P * m + P]
                                nc.tensor.matmul(
                                    h3p[:], w3ap,
                                    hh[k][:].bitcast(W13DT),
                                    start=(k == 0), stop=(k == NKH - 1))
                            s1 = slp.tile([P, TCH], F32,
                                          name=f"s1_{tq}{e}{m}", tag="s1")
                            nc.scalar.activation(s1[:], h1p[:], AF.Silu)
                            h3s = slp.tile([P, TCH], F32,
                                           name=f"h3s_{tq}{e}{m}", tag="h3s")
                            nc.vector.tensor_mul(h3s[:], h3p[:], wbs[:])
                            nc.vector.tensor_mul(gt[(e, m)][:], s1[:],
                                                 h3s[:])

                    # w2 phase: one stationary tile per output row-tile
                    for mo in range(NKH):
                        w2t = wp.tile([P, 2 * E * P], W2DT,
                                      name=f"w2_{tq}_{mo}", tag="w2s", bufs=2)
                        nc.sync.dma_start(w2t[:], w2T[mo])
                        mop = psM.tile([P, TCH], F32, name=f"mo{tq}{mo}",
                                       tag="mo", bufs=2)
                        first = True
                        for e in range(E):
                            for k2 in range(2):
                                j = 2 * e + k2
                                nc.tensor.matmul(
                                    mop[:], w2t[:, P * j:P * (j + 1)],
                                    gt[(e, k2)][:],
                                    start=first,
                                    stop=(e == E - 1 and k2 == 1))
                                first = False
                        mot = msb.tile([P, TCH], F32, name=f"mot{tq}{mo}",
                                       tag="mot")
                        nc.scalar.copy(mot[:], mop[:])
                        nc.sync.dma_start(
                            rs_in[tq][P * mo:P * (mo + 1), :], mot[:])
                    nc.gpsimd.collective_compute(
                        "ReduceScatter", ALU.add, replica_groups=RG,
                        ins=[rs_in[tq].opt()], outs=[rs_out[tq].opt()])

            # ======== final residual + norm ========
            with (
                tc.tile_pool(name="finsb", bufs=1) as fsb,
                tc.tile_pool(name="sqj", bufs=2) as sqj,
                tc.tile_pool(name="psJ", bufs=1, space="PSUM") as psJ,
            ):
                resid2 = fsb.tile([P, T], F32, tag="resid2")
                ss3 = fsb.tile([1, T], F32, tag="ss3")
                for n in range(NCH):
                    c0, c1 = TCH * n, TCH * (n + 1)
                    moe_sl = sqj.tile([P, TCH], F32, name=f"moesl{n}",
                                      tag="moesl")
                    nc.sync.dma_start(moe_sl[:], rs_out[n][:])
                    nc.vector.tensor_add(resid2[:, c0:c1], resid[:, c0:c1],
                                         moe_sl[:])
                    sq3 = sqj.tile([P, TCH], SDT, name=f"sq3_{n}", tag="sq3")
                    nc.scalar.activation(sq3[:], resid2[:, c0:c1], AF.Square)
                    ssp3 = psJ.tile([1, TCH], F32, name=f"ss3p{n}",
                                    tag="ss3p", bufs=2)
                    nc.tensor.matmul(ssp3[:], onec_t[:], sq3[:],
                                     start=True, stop=True)
                    nc.scalar.copy(ss3[:, c0:c1], ssp3[:])
                nc.sync.dma_start(ar3_in[:], ss3[:])
                nc.gpsimd.collective_compute(
                    "AllReduce", ALU.add, replica_groups=RG,
                    ins=[ar3_in.opt()], outs=[ar3_out.opt()])
                ss3g = fsb.tile([1, T], F32, tag="ss3g")
                nc.sync.dma_start(ss3g[:], ar3_out[:])
                outt = fsb.tile([P, T], F32, tag="outt")
                for n in range(NCH):
                    c0, c1 = TCH * n, TCH * (n + 1)
                    invt = rms_inv_chunk(ss3g[:, c0:c1], f"3_{n}")
                    bc = psJ.tile([P, TCH], F32, name=f"n3bc{n}", tag="n3bc",
                                  bufs=2)
                    nc.tensor.matmul(bc[:], oner_t[:], invt[:],
                                     start=True, stop=True)
                    nc.vector.tensor_mul(outt[:, c0:c1], bc[:],
                                         resid2[:, c0:c1])
                nc.vector.tensor_scalar_mul(outt[:], outt[:], wnext_t[:])
                nc.sync.dma_start(out_sl[:], outt[:])

    nc.compile()
    return nc


_W2NP = {"f32": np.float32, "f32r": np.float32, "bf16": ml_dtypes.bfloat16}


def _pack_pairs(a):
    """View a contiguous bf16 array as f32 pair-container (last dim halved)."""
    a = np.ascontiguousarray(a)
    if a.dtype == np.float32:
        return a
    return a.view(np.float32)


def _w13_flat(w13_c):
    """[E, H, 2*IS] f32 -> [E, 4, P, TCH] f32 pair-container of bf16 rows
    matching the SBUF tile layout: row (kk, p), columns (two, i)."""
    a = w13_c.reshape(E, 4, 2, P, 2 * IS)      # e kk two p i
    a = np.transpose(a, (0, 1, 3, 2, 4))       # e kk p two i
    a = np.ascontiguousarray(a.reshape(E, 4, P, 2 * TCH))
    a = a.astype(_W2NP[W13DT_NAME])
    if a.dtype != np.float32:
        a = np.ascontiguousarray(a).view(np.float32)
    return a


def _w2_layout(w2, c):
    """[E,H,I] -> [NKH, P, 2*E*P]: per output-row tile mo, stationary
    slices for (e, k2) laid contiguously."""
    w2c = w2[:, :, c * IS:(c + 1) * IS].transpose(0, 2, 1)  # [E, IS, H]
    out = np.zeros((NKH, P, 2 * E * P), np.float32)
    for e in range(E):
        for k2 in range(2):
            j = 2 * e + k2
            for mo in range(NKH):
                out[mo, :, P * j:P * (j + 1)] = \
                    w2c[e, P * k2:P * (k2 + 1), P * mo:P * (mo + 1)]
    return np.ascontiguousarray(out.astype(_W2NP[W2DT_NAME]))


def host_prep(inputs):
    """Build per-core in_maps from full inputs."""
    x = np.asarray(inputs["hidden_states"], np.float32)      # [T, H]
    pos = np.asarray(inputs["positions"])
    qkv_w = np.asarray(inputs["qkv_w"], np.float32)
    o_w = np.asarray(inputs["o_w"], np.float32)
    gate_w = np.asarray(inputs["gate_w"], np.float32)
    w1 = np.asarray(inputs["w1"], np.float32)
    w3 = np.asarray(inputs["w3"], np.float32)
    w2 = np.asarray(inputs["w2"], np.float32)
    nin = np.asarray(inputs["norm_in_w"], np.float32)
    npost = np.asarray(inputs["norm_post_w"], np.float32)
    nnext = np.asarray(inputs["norm_next_w"], np.float32)

    x_fm = np.ascontiguousarray(x.T)
    half = D // 2
    inv_freq = 1.0 / (THETA ** (np.arange(0, half, dtype=np.float32) * 2.0 / D))
    ang = pos.astype(np.float32)[:, None] * inv_freq[None, :]
    cos32 = np.cos(ang).T.astype(np.float32)
    sin32 = np.sin(ang).T.astype(np.float32)
    cos_q = np.ascontiguousarray(np.tile(cos32, (4, 1)))
    sin_q = np.ascontiguousarray(
        np.concatenate([-sin32, sin32, -sin32, sin32], 0))

    swap64 = np.zeros((64, 64), np.float32)
    swap64[0:32, 32:64] = np.eye(32, dtype=np.float32)
    swap64[32:64, 0:32] = np.eye(32, dtype=np.float32)
    qswap = np.zeros((P, P), np.float32)
    qswap[0:64, 0:64] = swap64
    qswap[64:128, 64:128] = swap64
    kdup = np.zeros((64, P), np.float32)
    kdup[np.arange(64), np.arange(64)] = 1.0
    kdup[np.arange(64), np.arange(64) + 64] = 1.0
    ident = np.eye(P, dtype=np.float32)
    ones_c = np.ones((P, 1), np.float32)
    ones_r = np.ones((1, P), np.float32)
    dmask = np.zeros((4, P, TCH), np.float32)
    pidx = np.arange(P)[:, None]
    fidx = np.arange(TCH)[None, :]
    for m in range(4):
        dmask[m] = np.where(fidx >= P * m + pidx, 0.0, NEG)

    gate_wT = np.ascontiguousarray((gate_w * npost[None, :]).T)   # [H, E]
    esel = np.zeros((E, E * P), np.float32)
    for e in range(E):
        esel[e, P * e:P * (e + 1)] = 1.0

    common = dict(x_fm=x_fm, cos_q=cos_q, sin_q=sin_q, qswap=qswap,
                  kswap=swap64, kdup=kdup, ident=ident, ones_c=ones_c,
                  ones_r=ones_r, dmask=dmask, esel=esel)

    scale = np.float32(D ** -0.5)
    in_maps = []
    for c in range(NC_N):
        q_rows = qkv_w[2 * c * D:(2 * c + 2) * D, :] * scale
        kv = c // 2
        k_rows = qkv_w[HQ * D + kv * D: HQ * D + (kv + 1) * D, :]
        v_rows = qkv_w[(HQ + HK) * D + kv * D: (HQ + HK) * D + (kv + 1) * D, :]
        wq = np.concatenate([q_rows, k_rows, v_rows], 0) * nin[None, :]
        w13_c = np.concatenate([
            (w1[:, c * IS:(c + 1) * IS, :] * npost[None, None, :]
             ).transpose(0, 2, 1),
            (w3[:, c * IS:(c + 1) * IS, :] * npost[None, None, :]
             ).transpose(0, 2, 1)], axis=2)              # [E, H, 512]
        m = dict(common)
        m.update(
            x_sl=np.ascontiguousarray(x_fm[P * c:P * (c + 1), :]),
            wqkv=np.ascontiguousarray(wq.T),
            wo=np.ascontiguousarray(o_w[P * c:P * (c + 1), :].T),
            gwl=np.ascontiguousarray(gate_wT[P * c:P * (c + 1), :]),
            w13=np.ascontiguousarray(w13_c),
            w2T=_w2_layout(w2, c),
            wnext=np.ascontiguousarray(
                nnext[P * c:P * (c + 1)].reshape(P, 1)),
        )
        in_maps.append(m)
    return in_maps


_NC_CACHE = None


def kernel(**inputs):
    global _NC_CACHE
    if _NC_CACHE is None:
        _NC_CACHE = build_program()
    nc = _NC_CACHE
    in_maps = host_prep(inputs)
    res = run_bass_kernel_spmd(nc, in_maps, core_ids=list(range(NC_N)))
    out_fm = np.concatenate([res.results[c]["out_sl"] for c in range(NC_N)], 0)
    return np.ascontiguousarray(out_fm.T)
